# revision 9
# baseline (speedup 1.0000x reference)
"""Trainium2 Bass kernel for nn_DualModalHyperGraph (dual-modal hypergraph conv).

Self-contained: builds one SPMD Bass/Tile program for 8 NeuronCores, shards
inputs on the host, runs via run_bass_kernel_spmd, reassembles the output.

Math (equivalent to the reference):
  sim_m = cols-normalized(mean_B feat_m) gram matrix    (per modality m)
  M_mk[n, r] = 1 iff r in top-(k+1) of row n            (two k per modality)
  Sigma_m = sum_k M_mk^T M_mk / (k+1)^2                 ([2048, 2048])
  A = D^-1/2 (blkdiag(Sigma_1, Sigma_2) + 1/4 [[I,I],[I,I]]) D^-1/2
  x1 = relu(A (x @ W1^T)); x2 = relu(A (x1 @ W2^T))

Sharding: core c = 4*m + s handles modality m, 512-row/col slab s. The
runtime supports only AllReduce/ReduceScatter collectives and no
registers/dynamic APs, so all per-core divergence is encoded in
host-provided flag tensors; mask exchange is an AllReduce of a
zero-padded stacked buffer, and the final layer is computed as partial
contributions summed by a ReduceScatter.
"""

import numpy as np

import concourse.bass as bass
import concourse.bacc as bacc
import concourse.mybir as mybir
import concourse.tile as tile
from concourse.bass_utils import run_bass_kernel_spmd
from concourse.masks import make_identity

P = 128
B = 4
N = 2048          # nodes per modality
NN = 2 * N
C = 64
F = 128
NT = 16           # 128-row tiles per modality
GT = 32           # global row tiles
MINVAL = -3.0e38
FP8 = mybir.dt.float8e4
F32 = mybir.dt.float32
F32R = mybir.dt.float32r
AL = mybir.AluOpType
AF = mybir.ActivationFunctionType

_CACHED_NC = None
LAST_EXEC_TIME_NS = None
LAST_RESULTS = None


def build_nc(f32r=False, drow=True):
    nc = bacc.Bacc("TRN2", target_bir_lowering=False, debug=False, num_devices=8)

    xT = nc.dram_tensor("xT", [C, B, NN], F32, kind="ExternalInput")
    xTm = nc.dram_tensor("xTm", [C, B, N], F32, kind="ExternalInput")
    xTmy = nc.dram_tensor("xTmy", [C, B, 512], F32, kind="ExternalInput")
    w1t = nc.dram_tensor("w1t", [C, F], F32, kind="ExternalInput")
    w2t = nc.dram_tensor("w2t", [F, F], F32, kind="ExternalInput")
    slotmask = nc.dram_tensor("slotmask", [P, 24], F32, kind="ExternalInput")
    cconst = nc.dram_tensor("cconst", [P, 8], F32, kind="ExternalInput")
    plflags = nc.dram_tensor("plflags", [P, 8], F32, kind="ExternalInput")
    qisel_in = nc.dram_tensor("qisel", [GT, P, 512], FP8, kind="ExternalInput")
    out_z = nc.dram_tensor("out_z", [B, 512, F], F32, kind="ExternalOutput")

    arin_m = nc.dram_tensor("arin_m", [2, GT, P, N], FP8)
    arout_m = nc.dram_tensor("arout_m", [2, GT, P, N], FP8, addr_space="Shared")
    arin_cs = nc.dram_tensor("arin_cs", [2, 2, N], F32)
    arout_cs = nc.dram_tensor("arout_cs", [2, 2, N], F32, addr_space="Shared")
    nsq_d = nc.dram_tensor("nsq_d", [1, N], F32)
    ninv_d = nc.dram_tensor("ninv_d", [1, N], F32)
    d_d = nc.dram_tensor("d_d", [2, N], F32)
    d2_d = nc.dram_tensor("d2_d", [2, N], F32)
    rsin = nc.dram_tensor("rsin", [8, 4, P, B * F], F32)
    rsout = nc.dram_tensor("rsout", [4, P, B * F], F32)

    ALLW = [list(range(8))]

    with tile.TileContext(nc) as tc:
      with tc.tile_pool(name="persist", bufs=1) as pp:
        cc = pp.tile([P, 8], F32)
        pf = pp.tile([P, 8], F32)
        sm = pp.tile([P, 24], F32)
        w1s = pp.tile([C, F], F32)
        w2s = pp.tile([F, F], F32)
        nc.sync.dma_start(cc[:], cconst[:])
        nc.sync.dma_start(pf[:], plflags[:])
        nc.sync.dma_start(sm[:], slotmask[:])
        nc.sync.dma_start(w1s[:], w1t[:])
        nc.sync.dma_start(w2s[:], w2t[:])
        ca0, ca1 = cc[:, 0:1], cc[:, 1:2]
        cb0, cb1 = cc[:, 2:3], cc[:, 3:4]
        f0_11, f1_11 = cc[0:1, 4:5], cc[0:1, 5:6]

        id128 = pp.tile([P, P], F32)
        make_identity(nc, id128[:])
        id32 = pp.tile([32, 32], F32)
        make_identity(nc, id32[:])
        ones8 = pp.tile([P, 1], FP8)
        nc.vector.memset(ones8[:], 1.0)
        idfl = pp.tile([P, 8, P], FP8)
        for k in range(8):
            nc.vector.tensor_scalar(idfl[:, k, :], id128[:], pf[:, k:k + 1], None, AL.mult)

        d_np = pp.tile([P, GT], F32)
        d2_np = pp.tile([P, GT], F32)
        d_own = pp.tile([P, 4], F32)
        d2_own = pp.tile([P, 4], F32)

        # ================= P1 + P2 + P3 =================
        with tc.tile_pool(name="psA", bufs=2, space="PSUM") as psA, \
             tc.tile_pool(name="topk", bufs=1) as tkp, \
             tc.tile_pool(name="tk2", bufs=2) as tk2:

            with tc.tile_pool(name="simprep", bufs=1) as sp:
                xTm_s = sp.tile([C, B, N], F32, tag="xTm_s")
                xTmy_s = sp.tile([C, B, 512], F32, tag="xTmy_s")
                nc.sync.dma_start(xTm_s[:], xTm[:])
                nc.sync.dma_start(xTmy_s[:], xTmy[:])

                fmy = tkp.tile([C, 512], F32)
                nc.vector.tensor_tensor(fmy[:], xTmy_s[:, 0], xTmy_s[:, 1], AL.add)
                nc.vector.tensor_tensor(fmy[:], fmy[:], xTmy_s[:, 2], AL.add)
                nc.vector.tensor_tensor(fmy[:], fmy[:], xTmy_s[:, 3], AL.add)
                nc.vector.tensor_scalar_mul(fmy[:], fmy[:], 0.25)

                fm = sp.tile([C, N], F32)
                nc.vector.tensor_tensor(fm[:], xTm_s[:, 0], xTm_s[:, 1], AL.add)
                nc.vector.tensor_tensor(fm[:], fm[:], xTm_s[:, 2], AL.add)
                nc.vector.tensor_tensor(fm[:], fm[:], xTm_s[:, 3], AL.add)
                nc.vector.tensor_scalar_mul(fm[:], fm[:], 0.25)

                fsq = sp.tile([C, N], F32, tag="xTmy_s")
                nc.vector.tensor_tensor(fsq[:], fm[:], fm[:], AL.mult)
                onesC = sp.tile([C, 1], F32)
                nc.vector.memset(onesC[:], 1.0)
                nsq_sb = sp.tile([1, N], F32)
                for ch in range(4):
                    ps = psA.tile([1, 512], F32, tag="ps1")
                    nc.tensor.matmul(ps[:], lhsT=onesC[:], rhs=fsq[:, ch * 512:(ch + 1) * 512],
                                     start=True, stop=True)
                    nc.scalar.activation(nsq_sb[:, ch * 512:(ch + 1) * 512], ps[:], AF.Copy)
                nc.sync.dma_start(nsq_d[:], nsq_sb[:])
                nperm = sp.tile([P, 16], F32)
                nc.sync.dma_start(nperm[:], nsq_d[:].rearrange("a (p j) -> p (a j)", p=P))
                nc.vector.tensor_scalar_max(nperm[:], nperm[:], 1e-24)
                nc.vector.reciprocal(nperm[:], nperm[:])
                nc.scalar.activation(nperm[:], nperm[:], AF.Sqrt)
                nc.sync.dma_start(ninv_d[:].rearrange("a (p j) -> p (a j)", p=P), nperm[:])
                ninv_rep = sp.tile([C, N], F32, tag="xTm_s")
                nc.sync.dma_start(ninv_rep[:], ninv_d[0:1, :].to_broadcast([C, N]))
                fhat = tkp.tile([C, N], F32)
                nc.vector.tensor_tensor(fhat[:], fm[:], ninv_rep[:], AL.mult)

            # ---- P2: sim rows, topk masks ----
            mA = tkp.tile([P, 4, N], FP8)
            mB = tkp.tile([P, 4, N], FP8)
            scr = tkp.tile([P, 8], F32)
            inv8 = tkp.tile([P, 8], F32)
            for t in range(4):
                sim_sb = tk2.tile([P, N], F32, tag="simsb")
                for ch in range(4):
                    ps = psA.tile([P, 512], F32, tag="ps512")
                    nc.tensor.matmul(ps[:], lhsT=fmy[:, t * P:(t + 1) * P],
                                     rhs=fhat[:, ch * 512:(ch + 1) * 512],
                                     start=True, stop=True)
                    nc.scalar.activation(sim_sb[:, ch * 512:(ch + 1) * 512], ps[:], AF.Copy)
                work = tk2.tile([P, N], F32, tag="work")
                src = sim_sb
                for r in range(3):
                    nc.vector.max(out=scr[:], in_=src[:])
                    nc.vector.tensor_tensor(scr[:], scr[:], sm[:, r * 8:(r + 1) * 8], AL.mult)
                    nc.vector.tensor_scalar(inv8[:], sm[:, r * 8:(r + 1) * 8],
                                            -MINVAL, MINVAL, AL.mult, AL.add)
                    nc.vector.tensor_tensor(scr[:], scr[:], inv8[:], AL.add)
                    nc.vector.match_replace(out=work[:], in_to_replace=scr[:],
                                            in_values=src[:], imm_value=MINVAL)
                    src = work
                    if r == 0:
                        nc.vector.tensor_tensor(mA[:, t, :], work[:], sim_sb[:], AL.not_equal)
                nc.vector.tensor_tensor(mB[:, t, :], work[:], sim_sb[:], AL.not_equal)

            # ---- placement into the mask-AllReduce input (flag-scaled) ----
            for qi, mq in enumerate((mA, mB)):
                mq2 = mq[:].rearrange("p t n -> p (t n)")
                for s in range(8):
                    sc = tk2.tile([P, 4 * N], FP8, tag=f"plc{s % 3}")
                    if s % 3 == 0:
                        nc.scalar.activation(sc[:], mq2, AF.Copy, scale=pf[:, s:s + 1])
                    elif s % 3 == 1:
                        nc.vector.tensor_scalar(sc[:], mq2, pf[:, s:s + 1], None, AL.mult)
                    else:
                        nc.scalar.activation(sc[:], mq2, AF.Copy, scale=pf[:, s:s + 1])
                    for t in range(4):
                        nc.sync.dma_start(arin_m[qi, s * 4 + t], sc[:, t * N:(t + 1) * N])

            # ---- colsums ----
            cs_sb = tkp.tile([1, 2, N], F32)
            for qi, mq in enumerate((mA, mB)):
                for ch in range(4):
                    ps = psA.tile([1, 512], F32, tag="ps1")
                    for t in range(4):
                        nc.tensor.matmul(ps[:], lhsT=ones8[:],
                                         rhs=mq[:, t, ch * 512:(ch + 1) * 512],
                                         start=(t == 0), stop=(t == 3))
                    nc.scalar.activation(cs_sb[:, qi, ch * 512:(ch + 1) * 512], ps[:], AF.Copy)
            cs2 = cs_sb[:].rearrange("a q n -> a (q n)")
            csc = tkp.tile([1, 2 * N], F32, tag="csc")
            nc.vector.tensor_scalar(csc[:], cs2, f0_11, None, AL.mult)
            nc.sync.dma_start(arin_cs[0:1].rearrange("a q n -> a (q n)"), csc[:])
            nc.vector.tensor_scalar(csc[:], cs2, f1_11, None, AL.mult)
            nc.sync.dma_start(arin_cs[1:2].rearrange("a q n -> a (q n)"), csc[:])

            # ---- collectives (same order on every core) ----
            nc.gpsimd.collective_compute("AllReduce", AL.add, replica_groups=ALLW,
                                         ins=[arin_m[0:1]], outs=[arout_m[0:1]])
            nc.gpsimd.collective_compute("AllReduce", AL.add, replica_groups=ALLW,
                                         ins=[arin_cs[:]], outs=[arout_cs[:]])
            nc.gpsimd.collective_compute("AllReduce", AL.add, replica_groups=ALLW,
                                         ins=[arin_m[1:2]], outs=[arout_m[1:2]])

            # ---- P3: degree vectors ----
            for m in range(2):
                ap_ = tk2.tile([P, 16], F32, tag="dva")
                bp_ = tk2.tile([P, 16], F32, tag="dvb")
                nc.sync.dma_start(ap_[:], arout_cs[m, 0:1, :].rearrange("a (p j) -> p (a j)", p=P))
                nc.sync.dma_start(bp_[:], arout_cs[m, 1:2, :].rearrange("a (p j) -> p (a j)", p=P))
                nc.vector.tensor_tensor(ap_[:], ap_[:], bp_[:], AL.add)
                nc.vector.tensor_scalar_add(ap_[:], ap_[:], 1.0)
                nc.vector.reciprocal(ap_[:], ap_[:])
                nc.sync.dma_start(d2_d[m:m + 1, :].rearrange("a (p j) -> p (a j)", p=P), ap_[:])
                nc.scalar.activation(ap_[:], ap_[:], AF.Sqrt)
                nc.sync.dma_start(d_d[m:m + 1, :].rearrange("a (p j) -> p (a j)", p=P), ap_[:])
            for dst, srcd in ((d_np, d_d), (d2_np, d2_d)):
                tr_in = tk2.tile([32, P], F32, tag="trin")
                nc.sync.dma_start(tr_in[:], srcd[:].rearrange("m (r c) -> (m r) c", r=16))
                pst = psA.tile([P, 32], F32, tag="pst")
                nc.tensor.transpose(pst[:], tr_in[:], id32[:])
                nc.scalar.activation(dst[:], pst[:], AF.Copy)
            dsel = tk2.tile([P, 4], F32, tag="dsel")
            for dst, srcT in ((d_own, d_np), (d2_own, d2_np)):
                nc.vector.memset(dst[:], 0.0)
                for g in range(8):
                    nc.vector.tensor_scalar(dsel[:], srcT[:, g * 4:(g + 1) * 4],
                                            pf[:, g:g + 1], None, AL.mult)
                    nc.vector.tensor_tensor(dst[:], dst[:], dsel[:], AL.add)

        # ================= P4: S-build directly into slab =================
        with tc.tile_pool(name="slabp", bufs=1) as slp:
            slab = slp.tile([P, GT, 512], F32)
            with tc.tile_pool(name="spool", bufs=1) as spl, \
                 tc.tile_pool(name="psS", bufs=2, space="PSUM") as psS:
                for qi in range(2):
                    mf = spl.tile([P, GT, N], FP8, tag="maskfull")
                    nc.sync.dma_start(mf[:], arout_m[qi].rearrange("g p n -> p g n"))
                    stg = spl.tile([P, GT, 512], FP8, tag="stage")
                    for g in range(GT):
                        mfl = g // NT
                        psg = psS.tile([P, 512], F32, tag="psg")
                        for j in range(4):
                            nc.tensor.matmul(psg[:], lhsT=idfl[:, 4 * mfl + j, :],
                                             rhs=mf[:, g, j * 512:(j + 1) * 512],
                                             start=(j == 0), stop=(j == 3))
                        nc.scalar.activation(stg[:, g, :], psg[:], AF.Copy)
                    w_top = ca0 if qi == 0 else cb0
                    w_bot = ca1 if qi == 0 else cb1
                    for mt in range(NT):
                        pss = psS.tile([P, 512], F32, tag="pss")
                        if drow:
                            for g2 in range(GT // 2):
                                nc.tensor.matmul(pss[:], lhsT=mf[:, 2 * g2:2 * g2 + 2, mt * P:(mt + 1) * P],
                                                 rhs=stg[:, 2 * g2:2 * g2 + 2, :],
                                                 start=(g2 == 0), stop=(g2 == GT // 2 - 1),
                                                 perf_mode=mybir.MatmulPerfMode.DoubleRow)
                        else:
                            for g in range(GT):
                                nc.tensor.matmul(pss[:], lhsT=mf[:, g, mt * P:(mt + 1) * P],
                                                 rhs=stg[:, g, :], start=(g == 0), stop=(g == GT - 1))
                        if qi == 0:
                            nc.scalar.activation(slab[:, mt, :], pss[:], AF.Copy, scale=w_top)
                            nc.vector.tensor_scalar(slab[:, NT + mt, :], pss[:], w_bot, None, AL.mult)
                        else:
                            tmp = spl.tile([P, 512], F32, tag="cbk")
                            nc.scalar.activation(tmp[:], pss[:], AF.Copy, scale=w_top)
                            nc.vector.tensor_tensor(slab[:, mt, :], slab[:, mt, :], tmp[:], AL.add)
                            tmp2 = spl.tile([P, 512], F32, tag="cbk2")
                            nc.vector.tensor_scalar(tmp2[:], pss[:], w_bot, None, AL.mult)
                            nc.vector.tensor_tensor(slab[:, NT + mt, :], slab[:, NT + mt, :],
                                                    tmp2[:], AL.add)

            # ---- P5: + quarter-identity J blocks ----
            with tc.tile_pool(name="qsp", bufs=1) as qsp:
                qs = qsp.tile([P, GT, 512], FP8)
                nc.sync.dma_start(qs[:], qisel_in[:].rearrange("g p n -> p g n"))
                s2 = slab[:].rearrange("p g n -> p (g n)")
                q2 = qs[:].rearrange("p g n -> p (g n)")
                nc.vector.tensor_tensor(s2[:, 0:NT * 512], s2[:, 0:NT * 512],
                                        q2[:, 0:NT * 512], AL.add)
                nc.vector.tensor_tensor(s2[:, NT * 512:], s2[:, NT * 512:],
                                        q2[:, NT * 512:], AL.add)

            # ================= P6: FM1 + AGG1 =================
            z1T = slp.tile([P, B, 512], F32, tag="z1T")
            with tc.tile_pool(name="fm1", bufs=1) as fmp, \
                 tc.tile_pool(name="xgp", bufs=3) as xgp, \
                 tc.tile_pool(name="psF", bufs=2, space="PSUM") as psF:
                u1 = fmp.tile([P, GT, B, F], F32)
                for g in range(GT):
                    xg = xgp.tile([C, B, P], F32, tag="xg")
                    nc.sync.dma_start(xg[:], xT[:, :, g * P:(g + 1) * P])
                    for b in range(B):
                        psy = psF.tile([P, F], F32, tag="psy")
                        nc.tensor.matmul(psy[:], lhsT=xg[:, b, :], rhs=w1s[:],
                                         start=True, stop=True)
                        nc.scalar.activation(u1[:, g, b, :], psy[:], AF.Copy,
                                             scale=d_np[:, g:g + 1])
                for b in range(B):
                    psz = psF.tile([P, 512], F32, tag="psz")
                    for g in range(GT):
                        if f32r:
                            nc.tensor.matmul(psz[:], lhsT=u1[:, g, b, :].bitcast(F32R),
                                             rhs=slab[:, g, :].bitcast(F32R),
                                             start=(g == 0), stop=(g == GT - 1))
                        else:
                            nc.tensor.matmul(psz[:], lhsT=u1[:, g, b, :], rhs=slab[:, g, :],
                                             start=(g == 0), stop=(g == GT - 1))
                    nc.scalar.activation(z1T[:, b, :], psz[:], AF.Relu)

            # ================= P8: slab^T, FM2, AGG2, RS, out =================
            with tc.tile_pool(name="slabTp", bufs=1) as sTp, \
                 tc.tile_pool(name="psT", bufs=2, space="PSUM") as psT:
                slabT = sTp.tile([P, 4, NN], F32)
                for g in range(GT):
                    for mt in range(4):
                        pstr = psT.tile([P, P], F32, tag="pstr")
                        nc.tensor.transpose(pstr[:], slab[:, g, mt * P:(mt + 1) * P], id128[:])
                        nc.scalar.activation(slabT[:, mt, g * P:(g + 1) * P], pstr[:], AF.Copy)
                u2 = sTp.tile([P, 4, B, F], F32, tag="u2")
                for mt in range(4):
                    for b in range(B):
                        psy2 = psT.tile([P, F], F32, tag="psy2")
                        nc.tensor.matmul(psy2[:], lhsT=z1T[:, b, mt * P:(mt + 1) * P],
                                         rhs=w2s[:], start=True, stop=True)
                        nc.scalar.activation(u2[:, mt, b, :], psy2[:], AF.Copy,
                                             scale=d2_own[:, mt:mt + 1])
                z2sb = sTp.tile([P, B * F], F32, tag="z2sb")
                for g in range(GT):
                    psz2 = psT.tile([P, B * F], F32, tag="psz2")
                    for kt in range(4):
                        if f32r:
                            nc.tensor.matmul(psz2[:], lhsT=slabT[:, kt, g * P:(g + 1) * P].bitcast(F32R),
                                             rhs=u2[:, kt].bitcast(F32R),
                                             start=(kt == 0), stop=(kt == 3))
                        else:
                            nc.tensor.matmul(psz2[:], lhsT=slabT[:, kt, g * P:(g + 1) * P],
                                             rhs=u2[:, kt], start=(kt == 0), stop=(kt == 3))
                    nc.scalar.activation(z2sb[:], psz2[:], AF.Copy)
                    nc.sync.dma_start(rsin[g // 4, g % 4], z2sb[:])

                nc.gpsimd.collective_compute("ReduceScatter", AL.add, replica_groups=ALLW,
                                             ins=[rsin[:]], outs=[rsout[:]])
                zf = sTp.tile([P, 4, B, F], F32, tag="zf")
                nc.sync.dma_start(zf[:], rsout[:].rearrange("t p (b f) -> p t b f", b=B))
                outsb = sTp.tile([P, 4, B, F], F32, tag="outsb")
                for mt in range(4):
                    nc.scalar.activation(outsb[:, mt], zf[:, mt], AF.Relu,
                                         scale=d_own[:, mt:mt + 1])
                for mt in range(4):
                    nc.sync.dma_start(
                        out_z[:, mt * P:(mt + 1) * P, :].rearrange("b p f -> p b f"),
                        outsb[:, mt])

    nc.compile()
    return nc


def _fp8(x):
    return x.astype(mybir.dt.np(FP8))


def _make_inputs(feat_mod1, feat_mod2, W1, W2):
    f1 = np.ascontiguousarray(np.asarray(feat_mod1), np.float32)
    f2 = np.ascontiguousarray(np.asarray(feat_mod2), np.float32)
    xT1 = np.ascontiguousarray(f1.transpose(2, 0, 1))
    xT2 = np.ascontiguousarray(f2.transpose(2, 0, 1))
    xT = np.ascontiguousarray(np.concatenate([xT1, xT2], axis=2))
    w1t = np.ascontiguousarray(np.asarray(W1, np.float32).T)
    w2t = np.ascontiguousarray(np.asarray(W2, np.float32).T)

    KS = {0: (7, 19), 1: (5, 13)}  # k+1 per modality
    in_maps = []
    for c in range(8):
        m, s = c // 4, c % 4
        xTm = xT1 if m == 0 else xT2
        xTmy = np.ascontiguousarray(xTm[:, :, s * 512:(s + 1) * 512])
        kA, kB = KS[m]
        slotm = np.zeros((P, 24), np.float32)
        slotm[:, 0:kA] = 1.0
        slotm[:, 8:16] = 1.0
        rem = kB - kA - 8
        if rem > 0:
            slotm[:, 16:16 + rem] = 1.0
        f0 = 1.0 if m == 0 else 0.0
        f1v = 1.0 - f0
        ccv = np.zeros((P, 8), np.float32)
        ccv[:, 0] = f0 / (kA * kA)   # ca0
        ccv[:, 1] = f1v / (kA * kA)  # ca1
        ccv[:, 2] = f0 / (kB * kB)   # cb0
        ccv[:, 3] = f1v / (kB * kB)  # cb1
        ccv[:, 4] = f0
        ccv[:, 5] = f1v
        plf = np.zeros((P, 8), np.float32)
        plf[:, c] = 1.0
        qis = np.zeros((GT, P, 512), np.float32)
        jj = np.arange(512)
        qis[s * 4 + jj // P, jj % P, jj] = 0.25
        qis[16 + s * 4 + jj // P, jj % P, jj] = 0.25
        in_maps.append({
            "xT": xT, "xTm": xTm, "xTmy": xTmy, "w1t": w1t, "w2t": w2t,
            "slotmask": slotm, "cconst": ccv, "plflags": plf, "qisel": _fp8(qis),
        })
    return in_maps


def kernel(feat_mod1, feat_mod2, W1, W2):
    global _CACHED_NC, LAST_EXEC_TIME_NS, LAST_RESULTS
    if _CACHED_NC is None:
        _CACHED_NC = build_nc()
    in_maps = _make_inputs(feat_mod1, feat_mod2, W1, W2)
    res = run_bass_kernel_spmd(_CACHED_NC, in_maps, list(range(8)))
    LAST_RESULTS = res
    LAST_EXEC_TIME_NS = getattr(res, "exec_time_ns", None)
    outs = [res.results[c]["out_z"] for c in range(8)]
    out1 = np.concatenate(outs[0:4], axis=1)
    out2 = np.concatenate(outs[4:8], axis=1)
    return out1, out2


# revision 10
# speedup vs baseline: 1.3291x; 1.3291x over previous
"""Trainium2 Bass kernel for nn_DualModalHyperGraph (dual-modal hypergraph conv).

Self-contained: builds one SPMD Bass/Tile program for 8 NeuronCores, shards
inputs on the host, runs via run_bass_kernel_spmd, reassembles the output.

Math (equivalent to the reference):
  sim_m = cols-normalized(mean_B feat_m) gram matrix    (per modality m)
  M_mk[n, r] = 1 iff r in top-(k+1) of row n            (two k per modality)
  Sigma_m = sum_k M_mk^T M_mk / (k+1)^2                 ([2048, 2048])
  A = D^-1/2 (blkdiag(Sigma_1, Sigma_2) + 1/4 [[I,I],[I,I]]) D^-1/2
  x1 = relu(A (x @ W1^T)); x2 = relu(A (x1 @ W2^T))

Sharding: core c = 4*m + s handles modality m, 512-row/col slab s. The
runtime supports only AllReduce/ReduceScatter collectives and no
registers/dynamic APs, so all per-core divergence is encoded in
host-provided flag tensors; mask exchange is an AllReduce of a
zero-padded stacked buffer, and the final layer is computed as partial
contributions summed by a ReduceScatter.
"""

import numpy as np

import concourse.bass as bass
import concourse.bacc as bacc
import concourse.mybir as mybir
import concourse.tile as tile
from concourse.bass_utils import run_bass_kernel_spmd
from concourse.masks import make_identity

P = 128
B = 4
N = 2048          # nodes per modality
NN = 2 * N
C = 64
F = 128
NT = 16           # 128-row tiles per modality
GT = 32           # global row tiles
MINVAL = -3.0e38
FP8 = mybir.dt.float8e4
F32 = mybir.dt.float32
F32R = mybir.dt.float32r
AL = mybir.AluOpType
AF = mybir.ActivationFunctionType

_CACHED_NC = None
LAST_EXEC_TIME_NS = None
LAST_RESULTS = None


def build_nc(f32r=False, drow=True):
    nc = bacc.Bacc("TRN2", target_bir_lowering=False, debug=False, num_devices=8)

    xT = nc.dram_tensor("xT", [C, B, NN], F32, kind="ExternalInput")
    xTm = nc.dram_tensor("xTm", [C, B, N], F32, kind="ExternalInput")
    xTmy = nc.dram_tensor("xTmy", [C, B, 512], F32, kind="ExternalInput")
    w1t = nc.dram_tensor("w1t", [C, F], F32, kind="ExternalInput")
    w2t = nc.dram_tensor("w2t", [F, F], F32, kind="ExternalInput")
    slotmask = nc.dram_tensor("slotmask", [P, 24], F32, kind="ExternalInput")
    cconst = nc.dram_tensor("cconst", [P, 8], F32, kind="ExternalInput")
    plflags = nc.dram_tensor("plflags", [P, 8], F32, kind="ExternalInput")
    qisel_in = nc.dram_tensor("qisel", [GT, P, 512], FP8, kind="ExternalInput")
    out_z = nc.dram_tensor("out_z", [B, 512, F], F32, kind="ExternalOutput")

    arin_m = nc.dram_tensor("arin_m", [2, GT, P, N], FP8)
    arout_m = nc.dram_tensor("arout_m", [2, GT, P, N], FP8, addr_space="Shared")
    arin_cs = nc.dram_tensor("arin_cs", [2, 2, N], F32)
    arout_cs = nc.dram_tensor("arout_cs", [2, 2, N], F32, addr_space="Shared")
    nsq_d = nc.dram_tensor("nsq_d", [1, N], F32)
    ninv_d = nc.dram_tensor("ninv_d", [1, N], F32)
    d_d = nc.dram_tensor("d_d", [2, N], F32)
    d2_d = nc.dram_tensor("d2_d", [2, N], F32)
    rsin = nc.dram_tensor("rsin", [8, 4, P, B * F], F32)
    rsout = nc.dram_tensor("rsout", [4, P, B * F], F32)

    ALLW = [list(range(8))]

    with tile.TileContext(nc) as tc:
      with tc.tile_pool(name="persist", bufs=1) as pp:
        cc = pp.tile([P, 8], F32)
        pf = pp.tile([P, 8], F32)
        sm = pp.tile([P, 24], F32)
        w1s = pp.tile([C, F], F32)
        w2s = pp.tile([F, F], F32)
        nc.sync.dma_start(cc[:], cconst[:])
        nc.sync.dma_start(pf[:], plflags[:])
        nc.sync.dma_start(sm[:], slotmask[:])
        nc.sync.dma_start(w1s[:], w1t[:])
        nc.sync.dma_start(w2s[:], w2t[:])
        ca0, ca1 = cc[:, 0:1], cc[:, 1:2]
        cb0, cb1 = cc[:, 2:3], cc[:, 3:4]
        f0_11, f1_11 = cc[0:1, 4:5], cc[0:1, 5:6]

        id128 = pp.tile([P, P], F32)
        make_identity(nc, id128[:])
        id32 = pp.tile([32, 32], F32)
        make_identity(nc, id32[:])
        ones8 = pp.tile([P, 1], FP8)
        nc.vector.memset(ones8[:], 1.0)
        idfl = pp.tile([P, 8, P], FP8)
        for k in range(8):
            nc.vector.tensor_scalar(idfl[:, k, :], id128[:], pf[:, k:k + 1], None, AL.mult)

        d_np = pp.tile([P, GT], F32)
        d2_np = pp.tile([P, GT], F32)
        d_own = pp.tile([P, 4], F32)
        d2_own = pp.tile([P, 4], F32)

        # ================= P1 + P2 + P3 =================
        with tc.tile_pool(name="psA", bufs=2, space="PSUM") as psA, \
             tc.tile_pool(name="topk", bufs=1) as tkp, \
             tc.tile_pool(name="tk2", bufs=2) as tk2:

            with tc.tile_pool(name="simprep", bufs=1) as sp:
                xTm_s = sp.tile([C, B, N], F32, tag="xTm_s")
                xTmy_s = sp.tile([C, B, 512], F32, tag="xTmy_s")
                nc.sync.dma_start(xTm_s[:], xTm[:])
                nc.sync.dma_start(xTmy_s[:], xTmy[:])

                fmy = tkp.tile([C, 512], F32)
                nc.vector.tensor_tensor(fmy[:], xTmy_s[:, 0], xTmy_s[:, 1], AL.add)
                nc.vector.tensor_tensor(fmy[:], fmy[:], xTmy_s[:, 2], AL.add)
                nc.vector.tensor_tensor(fmy[:], fmy[:], xTmy_s[:, 3], AL.add)
                nc.vector.tensor_scalar_mul(fmy[:], fmy[:], 0.25)

                fm = sp.tile([C, N], F32)
                nc.vector.tensor_tensor(fm[:], xTm_s[:, 0], xTm_s[:, 1], AL.add)
                nc.vector.tensor_tensor(fm[:], fm[:], xTm_s[:, 2], AL.add)
                nc.vector.tensor_tensor(fm[:], fm[:], xTm_s[:, 3], AL.add)
                nc.vector.tensor_scalar_mul(fm[:], fm[:], 0.25)

                fsq = sp.tile([C, N], F32, tag="xTmy_s")
                nc.vector.tensor_tensor(fsq[:], fm[:], fm[:], AL.mult)
                onesC = sp.tile([C, 1], F32)
                nc.vector.memset(onesC[:], 1.0)
                nsq_sb = sp.tile([1, N], F32)
                for ch in range(4):
                    ps = psA.tile([1, 512], F32, tag="ps1")
                    nc.tensor.matmul(ps[:], lhsT=onesC[:], rhs=fsq[:, ch * 512:(ch + 1) * 512],
                                     start=True, stop=True)
                    nc.scalar.activation(nsq_sb[:, ch * 512:(ch + 1) * 512], ps[:], AF.Copy)
                nc.sync.dma_start(nsq_d[:], nsq_sb[:])
                nperm = sp.tile([P, 16], F32)
                nc.sync.dma_start(nperm[:], nsq_d[:].rearrange("a (p j) -> p (a j)", p=P))
                nc.vector.tensor_scalar_max(nperm[:], nperm[:], 1e-24)
                nc.vector.reciprocal(nperm[:], nperm[:])
                nc.scalar.activation(nperm[:], nperm[:], AF.Sqrt)
                nc.sync.dma_start(ninv_d[:].rearrange("a (p j) -> p (a j)", p=P), nperm[:])
                ninv_rep = sp.tile([C, N], F32, tag="xTm_s")
                nc.sync.dma_start(ninv_rep[:], ninv_d[0:1, :].to_broadcast([C, N]))
                fhat = tkp.tile([C, N], F32)
                nc.vector.tensor_tensor(fhat[:], fm[:], ninv_rep[:], AL.mult)

            # ---- P2: sim rows, topk masks ----
            mA = tkp.tile([P, 4, N], FP8)
            mB = tkp.tile([P, 4, N], FP8)
            scr = tkp.tile([P, 8], F32)
            inv8 = tkp.tile([P, 8], F32)
            for t in range(4):
                sim_sb = tk2.tile([P, N], F32, tag="simsb")
                for ch in range(4):
                    ps = psA.tile([P, 512], F32, tag="ps512")
                    nc.tensor.matmul(ps[:], lhsT=fmy[:, t * P:(t + 1) * P],
                                     rhs=fhat[:, ch * 512:(ch + 1) * 512],
                                     start=True, stop=True)
                    nc.scalar.activation(sim_sb[:, ch * 512:(ch + 1) * 512], ps[:], AF.Copy)
                work = tk2.tile([P, N], F32, tag="work")
                src = sim_sb
                for r in range(3):
                    nc.vector.max(out=scr[:], in_=src[:])
                    nc.vector.tensor_tensor(scr[:], scr[:], sm[:, r * 8:(r + 1) * 8], AL.mult)
                    nc.vector.tensor_scalar(inv8[:], sm[:, r * 8:(r + 1) * 8],
                                            -MINVAL, MINVAL, AL.mult, AL.add)
                    nc.vector.tensor_tensor(scr[:], scr[:], inv8[:], AL.add)
                    nc.vector.match_replace(out=work[:], in_to_replace=scr[:],
                                            in_values=src[:], imm_value=MINVAL)
                    src = work
                    if r == 0:
                        nc.vector.tensor_tensor(mA[:, t, :], work[:], sim_sb[:], AL.not_equal)
                nc.vector.tensor_tensor(mB[:, t, :], work[:], sim_sb[:], AL.not_equal)

            # ---- placement into the mask-AllReduce input (flag-scaled) ----
            for qi, mq in enumerate((mA, mB)):
                mq2 = mq[:].rearrange("p t n -> p (t n)")
                for s in range(8):
                    sc = tk2.tile([P, 4 * N], FP8, tag=f"plc{s % 3}")
                    if s % 3 == 0:
                        nc.scalar.activation(sc[:], mq2, AF.Copy, scale=pf[:, s:s + 1])
                    elif s % 3 == 1:
                        nc.vector.tensor_scalar(sc[:], mq2, pf[:, s:s + 1], None, AL.mult)
                    else:
                        nc.scalar.activation(sc[:], mq2, AF.Copy, scale=pf[:, s:s + 1])
                    for t in range(4):
                        nc.sync.dma_start(arin_m[qi, s * 4 + t], sc[:, t * N:(t + 1) * N])

            # ---- colsums ----
            cs_sb = tkp.tile([1, 2, N], F32)
            for qi, mq in enumerate((mA, mB)):
                for ch in range(4):
                    ps = psA.tile([1, 512], F32, tag="ps1")
                    for t in range(4):
                        nc.tensor.matmul(ps[:], lhsT=ones8[:],
                                         rhs=mq[:, t, ch * 512:(ch + 1) * 512],
                                         start=(t == 0), stop=(t == 3))
                    nc.scalar.activation(cs_sb[:, qi, ch * 512:(ch + 1) * 512], ps[:], AF.Copy)
            cs2 = cs_sb[:].rearrange("a q n -> a (q n)")
            csc = tkp.tile([1, 2 * N], F32, tag="csc")
            nc.vector.tensor_scalar(csc[:], cs2, f0_11, None, AL.mult)
            nc.sync.dma_start(arin_cs[0:1].rearrange("a q n -> a (q n)"), csc[:])
            nc.vector.tensor_scalar(csc[:], cs2, f1_11, None, AL.mult)
            nc.sync.dma_start(arin_cs[1:2].rearrange("a q n -> a (q n)"), csc[:])

            # ---- collectives (same order on every core) ----
            nc.gpsimd.collective_compute("AllReduce", AL.add, replica_groups=ALLW,
                                         ins=[arin_cs[:]], outs=[arout_cs[:]])
            nc.gpsimd.collective_compute("AllReduce", AL.add, replica_groups=ALLW,
                                         ins=[arin_m[:]], outs=[arout_m[:]])

            # ---- P3: degree vectors ----
            for m in range(2):
                ap_ = tk2.tile([P, 16], F32, tag="dva")
                bp_ = tk2.tile([P, 16], F32, tag="dvb")
                nc.sync.dma_start(ap_[:], arout_cs[m, 0:1, :].rearrange("a (p j) -> p (a j)", p=P))
                nc.sync.dma_start(bp_[:], arout_cs[m, 1:2, :].rearrange("a (p j) -> p (a j)", p=P))
                nc.vector.tensor_tensor(ap_[:], ap_[:], bp_[:], AL.add)
                nc.vector.tensor_scalar_add(ap_[:], ap_[:], 1.0)
                nc.vector.reciprocal(ap_[:], ap_[:])
                nc.sync.dma_start(d2_d[m:m + 1, :].rearrange("a (p j) -> p (a j)", p=P), ap_[:])
                nc.scalar.activation(ap_[:], ap_[:], AF.Sqrt)
                nc.sync.dma_start(d_d[m:m + 1, :].rearrange("a (p j) -> p (a j)", p=P), ap_[:])
            for dst, srcd in ((d_np, d_d), (d2_np, d2_d)):
                tr_in = tk2.tile([32, P], F32, tag="trin")
                nc.sync.dma_start(tr_in[:], srcd[:].rearrange("m (r c) -> (m r) c", r=16))
                pst = psA.tile([P, 32], F32, tag="pst")
                nc.tensor.transpose(pst[:], tr_in[:], id32[:])
                nc.scalar.activation(dst[:], pst[:], AF.Copy)
            dsel = tk2.tile([P, 4], F32, tag="dsel")
            for dst, srcT in ((d_own, d_np), (d2_own, d2_np)):
                nc.vector.memset(dst[:], 0.0)
                for g in range(8):
                    nc.vector.tensor_scalar(dsel[:], srcT[:, g * 4:(g + 1) * 4],
                                            pf[:, g:g + 1], None, AL.mult)
                    nc.vector.tensor_tensor(dst[:], dst[:], dsel[:], AL.add)

        # ================= P4: S-build directly into slab =================
        with tc.tile_pool(name="slabp", bufs=1) as slp:
            slab = slp.tile([P, GT, 512], F32)
            with tc.tile_pool(name="spool", bufs=1) as spl, \
                 tc.tile_pool(name="psS", bufs=2, space="PSUM") as psS:
                for qi in range(2):
                    mf = spl.tile([P, GT, N], FP8, tag="maskfull")
                    nc.sync.dma_start(mf[:], arout_m[qi].rearrange("g p n -> p g n"))
                    stg = spl.tile([P, GT, 512], FP8, tag="stage")
                    for g in range(GT):
                        mfl = g // NT
                        psg = psS.tile([P, 512], F32, tag="psg")
                        for j in range(4):
                            nc.tensor.matmul(psg[:], lhsT=idfl[:, 4 * mfl + j, :],
                                             rhs=mf[:, g, j * 512:(j + 1) * 512],
                                             start=(j == 0), stop=(j == 3))
                        nc.scalar.activation(stg[:, g, :], psg[:], AF.Copy)
                    w_top = ca0 if qi == 0 else cb0
                    w_bot = ca1 if qi == 0 else cb1
                    for mt in range(NT):
                        pss = psS.tile([P, 512], F32, tag="pss")
                        if drow:
                            for g2 in range(GT // 2):
                                nc.tensor.matmul(pss[:], lhsT=mf[:, 2 * g2:2 * g2 + 2, mt * P:(mt + 1) * P],
                                                 rhs=stg[:, 2 * g2:2 * g2 + 2, :],
                                                 start=(g2 == 0), stop=(g2 == GT // 2 - 1),
                                                 perf_mode=mybir.MatmulPerfMode.DoubleRow)
                        else:
                            for g in range(GT):
                                nc.tensor.matmul(pss[:], lhsT=mf[:, g, mt * P:(mt + 1) * P],
                                                 rhs=stg[:, g, :], start=(g == 0), stop=(g == GT - 1))
                        if qi == 0:
                            nc.scalar.activation(slab[:, mt, :], pss[:], AF.Copy, scale=w_top)
                            nc.vector.tensor_scalar(slab[:, NT + mt, :], pss[:], w_bot, None, AL.mult)
                        else:
                            tmp = spl.tile([P, 512], F32, tag="cbk")
                            nc.scalar.activation(tmp[:], pss[:], AF.Copy, scale=w_top)
                            nc.vector.tensor_tensor(slab[:, mt, :], slab[:, mt, :], tmp[:], AL.add)
                            tmp2 = spl.tile([P, 512], F32, tag="cbk2")
                            nc.vector.tensor_scalar(tmp2[:], pss[:], w_bot, None, AL.mult)
                            nc.vector.tensor_tensor(slab[:, NT + mt, :], slab[:, NT + mt, :],
                                                    tmp2[:], AL.add)

            # ---- P5: + quarter-identity J blocks ----
            with tc.tile_pool(name="qsp", bufs=1) as qsp:
                qs = qsp.tile([P, GT, 512], FP8)
                nc.sync.dma_start(qs[:], qisel_in[:].rearrange("g p n -> p g n"))
                s2 = slab[:].rearrange("p g n -> p (g n)")
                q2 = qs[:].rearrange("p g n -> p (g n)")
                nc.vector.tensor_tensor(s2[:, 0:NT * 512], s2[:, 0:NT * 512],
                                        q2[:, 0:NT * 512], AL.add)
                nc.vector.tensor_tensor(s2[:, NT * 512:], s2[:, NT * 512:],
                                        q2[:, NT * 512:], AL.add)

            # ================= P6: FM1 + AGG1 =================
            z1T = slp.tile([P, B, 512], F32, tag="z1T")
            with tc.tile_pool(name="fm1", bufs=1) as fmp, \
                 tc.tile_pool(name="xgp", bufs=3) as xgp, \
                 tc.tile_pool(name="psF", bufs=2, space="PSUM") as psF:
                u1 = fmp.tile([P, GT, B, F], F32)
                for g in range(GT):
                    xg = xgp.tile([C, B, P], F32, tag="xg")
                    nc.sync.dma_start(xg[:], xT[:, :, g * P:(g + 1) * P])
                    for b in range(B):
                        psy = psF.tile([P, F], F32, tag="psy")
                        nc.tensor.matmul(psy[:], lhsT=xg[:, b, :], rhs=w1s[:],
                                         start=True, stop=True)
                        nc.scalar.activation(u1[:, g, b, :], psy[:], AF.Copy,
                                             scale=d_np[:, g:g + 1])
                for b in range(B):
                    psz = psF.tile([P, 512], F32, tag="psz")
                    for g in range(GT):
                        if f32r:
                            nc.tensor.matmul(psz[:], lhsT=u1[:, g, b, :].bitcast(F32R),
                                             rhs=slab[:, g, :].bitcast(F32R),
                                             start=(g == 0), stop=(g == GT - 1))
                        else:
                            nc.tensor.matmul(psz[:], lhsT=u1[:, g, b, :], rhs=slab[:, g, :],
                                             start=(g == 0), stop=(g == GT - 1))
                    nc.scalar.activation(z1T[:, b, :], psz[:], AF.Relu)

            # ================= P8: slab^T, FM2, AGG2, RS, out =================
            with tc.tile_pool(name="slabTp", bufs=1) as sTp, \
                 tc.tile_pool(name="psT", bufs=2, space="PSUM") as psT:
                slabT = sTp.tile([P, 4, NN], F32)
                for g in range(GT):
                    for mt in range(4):
                        pstr = psT.tile([P, P], F32, tag="pstr")
                        nc.tensor.transpose(pstr[:], slab[:, g, mt * P:(mt + 1) * P], id128[:])
                        nc.scalar.activation(slabT[:, mt, g * P:(g + 1) * P], pstr[:], AF.Copy)
                u2 = sTp.tile([P, 4, B, F], F32, tag="u2")
                for mt in range(4):
                    for b in range(B):
                        psy2 = psT.tile([P, F], F32, tag="psy2")
                        nc.tensor.matmul(psy2[:], lhsT=z1T[:, b, mt * P:(mt + 1) * P],
                                         rhs=w2s[:], start=True, stop=True)
                        nc.scalar.activation(u2[:, mt, b, :], psy2[:], AF.Copy,
                                             scale=d2_own[:, mt:mt + 1])
                z2sb = sTp.tile([P, B * F], F32, tag="z2sb")
                for g in range(GT):
                    psz2 = psT.tile([P, B * F], F32, tag="psz2")
                    for kt in range(4):
                        if f32r:
                            nc.tensor.matmul(psz2[:], lhsT=slabT[:, kt, g * P:(g + 1) * P].bitcast(F32R),
                                             rhs=u2[:, kt].bitcast(F32R),
                                             start=(kt == 0), stop=(kt == 3))
                        else:
                            nc.tensor.matmul(psz2[:], lhsT=slabT[:, kt, g * P:(g + 1) * P],
                                             rhs=u2[:, kt], start=(kt == 0), stop=(kt == 3))
                    nc.scalar.activation(z2sb[:], psz2[:], AF.Copy)
                    nc.sync.dma_start(rsin[g // 4, g % 4], z2sb[:])

                nc.gpsimd.collective_compute("ReduceScatter", AL.add, replica_groups=ALLW,
                                             ins=[rsin[:]], outs=[rsout[:]])
                zf = sTp.tile([P, 4, B, F], F32, tag="zf")
                nc.sync.dma_start(zf[:], rsout[:].rearrange("t p (b f) -> p t b f", b=B))
                outsb = sTp.tile([P, 4, B, F], F32, tag="outsb")
                for mt in range(4):
                    nc.scalar.activation(outsb[:, mt], zf[:, mt], AF.Relu,
                                         scale=d_own[:, mt:mt + 1])
                for mt in range(4):
                    nc.sync.dma_start(
                        out_z[:, mt * P:(mt + 1) * P, :].rearrange("b p f -> p b f"),
                        outsb[:, mt])

    nc.compile()
    return nc


def _fp8(x):
    return x.astype(mybir.dt.np(FP8))


def _make_inputs(feat_mod1, feat_mod2, W1, W2):
    f1 = np.ascontiguousarray(np.asarray(feat_mod1), np.float32)
    f2 = np.ascontiguousarray(np.asarray(feat_mod2), np.float32)
    xT1 = np.ascontiguousarray(f1.transpose(2, 0, 1))
    xT2 = np.ascontiguousarray(f2.transpose(2, 0, 1))
    xT = np.ascontiguousarray(np.concatenate([xT1, xT2], axis=2))
    w1t = np.ascontiguousarray(np.asarray(W1, np.float32).T)
    w2t = np.ascontiguousarray(np.asarray(W2, np.float32).T)

    KS = {0: (7, 19), 1: (5, 13)}  # k+1 per modality
    in_maps = []
    for c in range(8):
        m, s = c // 4, c % 4
        xTm = xT1 if m == 0 else xT2
        xTmy = np.ascontiguousarray(xTm[:, :, s * 512:(s + 1) * 512])
        kA, kB = KS[m]
        slotm = np.zeros((P, 24), np.float32)
        slotm[:, 0:kA] = 1.0
        slotm[:, 8:16] = 1.0
        rem = kB - kA - 8
        if rem > 0:
            slotm[:, 16:16 + rem] = 1.0
        f0 = 1.0 if m == 0 else 0.0
        f1v = 1.0 - f0
        ccv = np.zeros((P, 8), np.float32)
        ccv[:, 0] = f0 / (kA * kA)   # ca0
        ccv[:, 1] = f1v / (kA * kA)  # ca1
        ccv[:, 2] = f0 / (kB * kB)   # cb0
        ccv[:, 3] = f1v / (kB * kB)  # cb1
        ccv[:, 4] = f0
        ccv[:, 5] = f1v
        plf = np.zeros((P, 8), np.float32)
        plf[:, c] = 1.0
        qis = np.zeros((GT, P, 512), np.float32)
        jj = np.arange(512)
        qis[s * 4 + jj // P, jj % P, jj] = 0.25
        qis[16 + s * 4 + jj // P, jj % P, jj] = 0.25
        in_maps.append({
            "xT": xT, "xTm": xTm, "xTmy": xTmy, "w1t": w1t, "w2t": w2t,
            "slotmask": slotm, "cconst": ccv, "plflags": plf, "qisel": _fp8(qis),
        })
    return in_maps


def kernel(feat_mod1, feat_mod2, W1, W2):
    global _CACHED_NC, LAST_EXEC_TIME_NS, LAST_RESULTS
    if _CACHED_NC is None:
        _CACHED_NC = build_nc()
    in_maps = _make_inputs(feat_mod1, feat_mod2, W1, W2)
    res = run_bass_kernel_spmd(_CACHED_NC, in_maps, list(range(8)))
    LAST_RESULTS = res
    LAST_EXEC_TIME_NS = getattr(res, "exec_time_ns", None)
    outs = [res.results[c]["out_z"] for c in range(8)]
    out1 = np.concatenate(outs[0:4], axis=1)
    out2 = np.concatenate(outs[4:8], axis=1)
    return out1, out2
